# revision 14
# baseline (speedup 1.0000x reference)
"""Trainium2 Bass kernel for ChannelHyperedgeInteraction.

Contract: kernel(**inputs) takes the FULL (unsharded) numpy inputs as
produced by setup_inputs() and returns the FULL [16,128,1024] output.

Strategy (hardcoded for B=16, C=128, L=256, d=1024, d_ff=4096, 8 cores):
  * Data-parallel over B: 2 samples per NeuronCore, weights replicated.
  * Masked mean-pool runs on the tensor engine: for each channel c the
    stationary operand is a [128l, 32] zero tile whose column (c%32) is
    that channel's mask chunk, so 32-channel groups accumulate straight
    into PSUM partitions 32a..32a+32 and E materializes as [c, d] with a
    single fused scale (1/count) evacuation per sample.
  * Transformer phase computes qT/kT (feature-major) and v/hT
    (row-major) so every GEMM's stationary operand comes directly out of
    the previous step; gelu+bias fused on the scalar engine; LayerNorm
    via bn_stats/bn_aggr; per-sample [128,128] softmax with fused
    exp+row-sum.
"""

import sys

import numpy as np

if "/opt/trn_rl_repo" not in sys.path:
    sys.path.insert(0, "/opt/trn_rl_repo")

import concourse.bass as bass
import concourse.tile as tile
from concourse import bacc
from concourse import mybir
from concourse.masks import make_identity

F32 = mybir.dt.float32
P = 128
EPS = 1e-5
N_CORES = 8

Act = mybir.ActivationFunctionType
Alu = mybir.AluOpType
AX = mybir.AxisListType


def build_program(S=2, C=128, L=256, D=1024, DFF=4096, use_gelu=True,
                  debug=False):
    """Emit the per-core SPMD Bass/Tile program.

    S: samples per core. C: channels (<=128, multiple of 32).
    """
    assert C <= P and C % 32 == 0 and L % P == 0 and D % 512 == 0
    NKC = D // P          # contraction chunks over d
    NLC = L // P          # chunks over L
    NFF = DFF // P        # chunks over d_ff
    NG = C // 32          # 32-channel pooling groups
    NH = D // 512         # 512-wide output halves
    R = S * C             # total pooled rows per core

    nc = bacc.Bacc()

    x_d = nc.dram_tensor("x", [S, C, L, D], F32, kind="ExternalInput")
    mask_d = nc.dram_tensor("mask", [S, C, L], F32, kind="ExternalInput")
    wq_d = nc.dram_tensor("Wq", [D, D], F32, kind="ExternalInput")
    bq_d = nc.dram_tensor("bq", [D], F32, kind="ExternalInput")
    wk_d = nc.dram_tensor("Wk", [D, D], F32, kind="ExternalInput")
    bk_d = nc.dram_tensor("bk", [D], F32, kind="ExternalInput")
    wv_d = nc.dram_tensor("Wv", [D, D], F32, kind="ExternalInput")
    bv_d = nc.dram_tensor("bv", [D], F32, kind="ExternalInput")
    wo_d = nc.dram_tensor("Wo", [D, D], F32, kind="ExternalInput")
    bo_d = nc.dram_tensor("bo", [D], F32, kind="ExternalInput")
    w1_d = nc.dram_tensor("W1", [D, DFF], F32, kind="ExternalInput")
    b1_d = nc.dram_tensor("b1", [DFF], F32, kind="ExternalInput")
    w2_d = nc.dram_tensor("W2", [DFF, D], F32, kind="ExternalInput")
    b2_d = nc.dram_tensor("b2", [D], F32, kind="ExternalInput")
    g1_d = nc.dram_tensor("g1", [D], F32, kind="ExternalInput")
    be1_d = nc.dram_tensor("be1", [D], F32, kind="ExternalInput")
    g2_d = nc.dram_tensor("g2", [D], F32, kind="ExternalInput")
    be2_d = nc.dram_tensor("be2", [D], F32, kind="ExternalInput")
    y_d = nc.dram_tensor("y", [S, C, D], F32, kind="ExternalOutput")
    if debug:
        dbg_e = nc.dram_tensor("dbg_e", [S, C, D], F32, kind="ExternalOutput")
        dbg_mod = nc.dram_tensor("dbg_mod", [S, C, C], F32, kind="ExternalOutput")
        dbg_v = nc.dram_tensor("dbg_v", [S, C, D], F32, kind="ExternalOutput")
        dbg_attn = nc.dram_tensor("dbg_attn", [S, C, C], F32, kind="ExternalOutput")
        dbg_x1 = nc.dram_tensor("dbg_x1", [S, C, D], F32, kind="ExternalOutput")
        dbg_e1 = nc.dram_tensor("dbg_e1", [S, C, D], F32, kind="ExternalOutput")
        dbg_x2 = nc.dram_tensor("dbg_x2", [S, C, D], F32, kind="ExternalOutput")

    gelu_func = Act.Gelu if use_gelu else Act.Identity

    with tile.TileContext(nc) as tc:
        from contextlib import ExitStack

        with ExitStack() as ctx:
            const = ctx.enter_context(tc.tile_pool(name="const", bufs=1))
            xpool = ctx.enter_context(tc.tile_pool(name="xpool", bufs=2))
            wpool = ctx.enter_context(tc.tile_pool(name="wpool", bufs=4))
            work = ctx.enter_context(tc.tile_pool(name="work", bufs=2))
            hpool = ctx.enter_context(tc.tile_pool(name="hpool", bufs=3))
            small = ctx.enter_context(tc.tile_pool(name="small", bufs=4))
            ps_big = ctx.enter_context(
                tc.tile_pool(name="ps_big", bufs=2, space="PSUM"))
            ps_med = ctx.enter_context(
                tc.tile_pool(name="ps_med", bufs=2, space="PSUM"))
            ps_sm = ctx.enter_context(
                tc.tile_pool(name="ps_sm", bufs=2, space="PSUM"))

            # ---- constants -------------------------------------------------
            ident = const.tile([P, P], F32)
            make_identity(nc, ident[:])
            eps_t = const.tile([P, 1], F32)
            nc.vector.memset(eps_t[:], EPS)
            ones_t = const.tile([P, P], F32)
            nc.vector.memset(ones_t[:], 1.0)

            g1b = const.tile([P, D], F32)
            nc.sync.dma_start(g1b[:], g1_d[None, :].to_broadcast((P, D)))
            be1b = const.tile([P, D], F32)
            nc.sync.dma_start(be1b[:], be1_d[None, :].to_broadcast((P, D)))
            g2b = const.tile([P, D], F32)
            nc.sync.dma_start(g2b[:], g2_d[None, :].to_broadcast((P, D)))
            be2b = const.tile([P, D], F32)
            nc.sync.dma_start(be2b[:], be2_d[None, :].to_broadcast((P, D)))
            bob = const.tile([P, D], F32)
            nc.sync.dma_start(bob[:], bo_d[None, :].to_broadcast((P, D)))
            b2b = const.tile([P, D], F32)
            nc.sync.dma_start(b2b[:], b2_d[None, :].to_broadcast((P, D)))

            bqt = const.tile([P, NKC], F32)
            nc.sync.dma_start(bqt[:], bq_d.rearrange("(kc p) -> p kc", p=P))
            bkt = const.tile([P, NKC], F32)
            nc.sync.dma_start(bkt[:], bk_d.rearrange("(kc p) -> p kc", p=P))
            bvt = const.tile([P, NKC], F32)
            nc.sync.dma_start(bvt[:], bv_d.rearrange("(kc p) -> p kc", p=P))
            b1t = const.tile([P, NFF], F32)
            nc.sync.dma_start(b1t[:], b1_d.rearrange("(f p) -> p f", p=P))

            # ---- mask prep -------------------------------------------------
            mask_sb = const.tile([P, S, L], F32)
            total = const.tile([P, S], F32)
            inv_cnt = const.tile([P, S], F32)
            maskT = const.tile([P, S, NLC, C], F32)
            trow_sb = const.tile([P, S, C], F32)   # only partition 0 used
            mod_sb = const.tile([P, S, C], F32)

            for s in range(S):
                nc.sync.dma_start(mask_sb[:C, s, :], mask_d[s])
                nc.vector.reduce_sum(total[:C, s:s + 1], mask_sb[:C, s, :],
                                     axis=AX.X)
                cnt = small.tile([P, 1], F32, tag="cnt")
                nc.vector.tensor_scalar_max(cnt[:C], total[:C, s:s + 1], 1.0)
                nc.vector.reciprocal(inv_cnt[:C, s:s + 1], cnt[:C])
                for k in range(NLC):
                    tp = ps_sm.tile([P, C], F32, tag="sm")
                    nc.tensor.matmul(tp[:, :C],
                                     mask_sb[:C, s, k * P:(k + 1) * P],
                                     ident[:C, :C])
                    nc.vector.tensor_copy(maskT[:, s, k, :], tp[:, :C])

            # ---- attention mask-overlap modulation -------------------------
            for s in range(S):
                tr_ps = ps_sm.tile([P, C], F32, tag="sm")
                for k in range(NLC):
                    nc.tensor.matmul(tr_ps[0:1, :C], ones_t[:, 0:1],
                                     maskT[:, s, k, :],
                                     start=(k == 0), stop=(k == NLC - 1))
                nc.vector.tensor_copy(trow_sb[0:1, s, :], tr_ps[0:1, :C])
                # broadcast total_row across partitions: ones[:,0:1] x trow
                trb_ps = ps_sm.tile([P, C], F32, tag="sm")
                nc.tensor.matmul(trb_ps[:C, :C], ones_t[0:1, :C],
                                 trow_sb[0:1, s, :])
                joint_ps = ps_sm.tile([P, C], F32, tag="sm")
                for k in range(NLC):
                    nc.tensor.matmul(joint_ps[:C, :C], maskT[:, s, k, :],
                                     maskT[:, s, k, :],
                                     start=(k == 0), stop=(k == NLC - 1))
                tpm = work.tile([P, C], F32, tag="tpm")
                nc.vector.tensor_scalar(tpm[:C], trb_ps[:C, :C],
                                        total[:C, s:s + 1], 1.0,
                                        op0=Alu.add, op1=Alu.max)
                nc.vector.reciprocal(tpm[:C], tpm[:C])
                nc.vector.tensor_mul(tpm[:C], joint_ps[:C, :C], tpm[:C])
                # S_final = S_raw*(0.5 + joint/tp)/sqrt(d)
                sc = 1.0 / float(np.sqrt(D))
                nc.vector.tensor_scalar(mod_sb[:C, s, :], tpm[:C],
                                        sc, 0.5 * sc,
                                        op0=Alu.mult, op1=Alu.add)
                if debug:
                    nc.sync.dma_start(dbg_mod[s], mod_sb[:C, s, :])

            # ---- pooling: E[c,d] = sum_l mask*x, then *1/count -------------
            dall = const.tile([P, NLC, NG, 32 * 32], F32)
            nc.vector.memset(dall[:], 0.0)
            e_sb = const.tile([P, S, D], F32)
            for s in range(S):
                for k in range(NLC):
                    for a in range(NG):
                        blk = dall[:, k, a, :]
                        diag = bass.AP(tensor=blk.tensor, offset=blk.offset,
                                       ap=[blk.ap[0], [33, 32]])
                        nc.vector.tensor_copy(
                            diag, maskT[:, s, k, a * 32:(a + 1) * 32])
                e_ps = ps_big.tile([P, D], F32, tag="big")
                for t in range(C // 2):
                    xt = xpool.tile([P, 2, NLC, D], F32, tag="xt")
                    nc.sync.dma_start(
                        xt[:],
                        x_d[s, 2 * t:2 * t + 2].rearrange(
                            "ci (k p) d -> p ci k d", p=P))
                    for ci in range(2):
                        c = 2 * t + ci
                        a, r = c // 32, c % 32
                        for k in range(NLC):
                            for h in range(NH):
                                nc.tensor.matmul(
                                    e_ps[32 * a:32 * a + 32,
                                         h * 512:(h + 1) * 512],
                                    dall[:, k, a, 32 * r:32 * r + 32],
                                    xt[:, ci, k, h * 512:(h + 1) * 512],
                                    start=(c % 32 == 0 and k == 0),
                                    stop=(c % 32 == 31 and k == NLC - 1),
                                    skip_group_check=True,
                                    tile_position=(0, 32 * a))
                nc.vector.tensor_scalar_mul(e_sb[:C, s, :], e_ps[:C, :],
                                            inv_cnt[:C, s:s + 1])
                if debug:
                    nc.sync.dma_start(dbg_e[s], e_sb[:C, s, :])

            # ---- ET --------------------------------------------------------
            et = const.tile([P, NKC, R], F32)
            for s in range(S):
                for kc in range(NKC):
                    tp = ps_sm.tile([P, C], F32, tag="sm")
                    nc.tensor.matmul(tp[:, :C],
                                     e_sb[:C, s, kc * P:(kc + 1) * P],
                                     ident[:C, :C])
                    nc.vector.tensor_copy(et[:, kc, s * C:(s + 1) * C],
                                          tp[:, :C])

            # ---- qT / kT ---------------------------------------------------
            qt = const.tile([P, NKC, R], F32)
            kt = const.tile([P, NKC, R], F32)
            for w_d, out_t, b_t in ((wq_d, qt, bqt), (wk_d, kt, bkt)):
                for n in range(NKC):
                    wt = wpool.tile([P, NKC, P], F32, tag="w")
                    nc.sync.dma_start(
                        wt[:], w_d[:, n * P:(n + 1) * P].rearrange(
                            "(kc p) f -> p kc f", p=P))
                    qp = ps_med.tile([P, R], F32, tag="mm")
                    for kc in range(NKC):
                        nc.tensor.matmul(qp[:], wt[:, kc, :], et[:, kc, :],
                                         start=(kc == 0),
                                         stop=(kc == NKC - 1))
                    nc.vector.tensor_scalar_add(out_t[:, n, :], qp[:],
                                                b_t[:, n:n + 1])

            # ---- v ---------------------------------------------------------
            v_sb = const.tile([P, S, D], F32)
            v_ps = [ps_big.tile([P, D], F32, tag="big", name=f"v_ps{s}")
                    for s in range(S)]
            for kc in range(NKC):
                wt = wpool.tile([P, D], F32, tag="w")
                nc.sync.dma_start(wt[:], wv_d[kc * P:(kc + 1) * P, :])
                for s in range(S):
                    for h in range(NH):
                        nc.tensor.matmul(
                            v_ps[s][:C, h * 512:(h + 1) * 512],
                            et[:, kc, s * C:(s + 1) * C],
                            wt[:, h * 512:(h + 1) * 512],
                            start=(kc == 0), stop=(kc == NKC - 1),
                            skip_group_check=True)
            for s in range(S):
                nc.vector.tensor_copy(v_sb[:C, s, :], v_ps[s][:C, :])
                if debug:
                    nc.sync.dma_start(dbg_v[s], v_sb[:C, s, :])

            # ---- attention per sample -------------------------------------
            aot = const.tile([P, S, NKC, C], F32)
            for s in range(S):
                s_ps = ps_sm.tile([P, C], F32, tag="sm")
                for n in range(NKC):
                    nc.tensor.matmul(s_ps[:C, :C],
                                     qt[:, n, s * C:(s + 1) * C],
                                     kt[:, n, s * C:(s + 1) * C],
                                     start=(n == 0), stop=(n == NKC - 1))
                smod = work.tile([P, C], F32, tag="attn")
                nc.vector.tensor_mul(smod[:C], s_ps[:C, :C], mod_sb[:C, s, :])
                negmax = small.tile([P, 1], F32, tag="nm")
                nc.vector.reduce_max(negmax[:C], smod[:C], axis=AX.X,
                                     negate=True)
                exps = work.tile([P, C], F32, tag="attn")
                sumexp = small.tile([P, 1], F32, tag="se")
                nc.scalar.activation(exps[:C], smod[:C], Act.Exp,
                                     bias=negmax[:C], accum_out=sumexp[:C])
                nc.vector.reciprocal(sumexp[:C], sumexp[:C])
                nc.vector.tensor_scalar_mul(exps[:C], exps[:C], sumexp[:C])
                if debug:
                    nc.sync.dma_start(dbg_attn[s], exps[:C])
                at_ps = ps_sm.tile([P, C], F32, tag="sm")
                nc.tensor.matmul(at_ps[:C, :C], exps[:C], ident[:C, :C])
                attnT = work.tile([P, C], F32, tag="attnT")
                nc.vector.tensor_copy(attnT[:C], at_ps[:C, :C])
                # attn-weighted values, transposed: aoT[d,c] (+bv fused: rows
                # of attn sum to 1 so bv passes through exactly)
                for dc in range(NKC):
                    ao_ps = ps_sm.tile([P, C], F32, tag="sm")
                    nc.tensor.matmul(ao_ps[:, :C],
                                     v_sb[:C, s, dc * P:(dc + 1) * P],
                                     attnT[:C])
                    nc.vector.tensor_scalar_add(aot[:, s, dc, :],
                                                ao_ps[:, :C],
                                                bvt[:, dc:dc + 1])

            # ---- O-projection + residual + LN1 ----------------------------
            def layer_norm(dst, src, g_b, be_b):
                stats = small.tile([P, D // 512, 6], F32, tag="lnst")
                for sub in range(D // 512):
                    nc.vector.bn_stats(stats[:C, sub, :],
                                       src[:, sub * 512:(sub + 1) * 512])
                mv = small.tile([P, 2], F32, tag="lnmv")
                nc.vector.bn_aggr(mv[:C, :], stats[:C, :, :])
                rstd = small.tile([P, 1], F32, tag="lnr")
                nc.scalar.activation(rstd[:C], mv[:C, 1:2], Act.Sqrt,
                                     bias=eps_t[:C])
                nc.vector.reciprocal(rstd[:C], rstd[:C])
                nc.vector.tensor_scalar(dst, src, mv[:C, 0:1], rstd[:C],
                                        op0=Alu.subtract, op1=Alu.mult)
                nc.vector.tensor_mul(dst, dst, g_b[:C])
                nc.vector.tensor_add(dst, dst, be_b[:C])

            e1 = const.tile([P, S, D], F32)
            r_ps = [ps_big.tile([P, D], F32, tag="big", name=f"r_ps{s}")
                    for s in range(S)]
            for dc in range(NKC):
                wt = wpool.tile([P, D], F32, tag="w")
                nc.sync.dma_start(wt[:], wo_d[dc * P:(dc + 1) * P, :])
                for s in range(S):
                    for h in range(NH):
                        nc.tensor.matmul(
                            r_ps[s][:C, h * 512:(h + 1) * 512],
                            aot[:, s, dc, :], wt[:, h * 512:(h + 1) * 512],
                            start=(dc == 0), stop=(dc == NKC - 1),
                            skip_group_check=True)
            for s in range(S):
                x1 = work.tile([P, D], F32, tag="x")
                nc.vector.tensor_add(x1[:C], e_sb[:C, s, :], r_ps[s][:C, :])
                nc.vector.tensor_add(x1[:C], x1[:C], bob[:C])
                if debug:
                    nc.sync.dma_start(dbg_x1[s], x1[:C])
                layer_norm(e1[:C, s, :], x1[:C], g1b, be1b)
                if debug:
                    nc.sync.dma_start(dbg_e1[s], e1[:C, s, :])

            # ---- E1T -------------------------------------------------------
            e1t = const.tile([P, NKC, R], F32)
            for s in range(S):
                for kc in range(NKC):
                    tp = ps_sm.tile([P, C], F32, tag="sm")
                    nc.tensor.matmul(tp[:, :C],
                                     e1[:C, s, kc * P:(kc + 1) * P],
                                     ident[:C, :C])
                    nc.vector.tensor_copy(e1t[:, kc, s * C:(s + 1) * C],
                                          tp[:, :C])

            # ---- FFN -------------------------------------------------------
            o2_ps = [ps_big.tile([P, D], F32, tag="big", name=f"o2_ps{s}")
                     for s in range(S)]
            for s in range(S):
                for h in range(NH):
                    nc.tensor.matmul(o2_ps[s][0:1, h * 512:h * 512 + 1],
                                     ones_t[:, 0:1], ones_t[:, 0:1],
                                     skip_group_check=True)
            for f in range(NFF):
                wt = wpool.tile([P, NKC, P], F32, tag="w")
                nc.sync.dma_start(
                    wt[:], w1_d[:, f * P:(f + 1) * P].rearrange(
                        "(kc p) g -> p kc g", p=P))
                h_ps = ps_med.tile([P, R], F32, tag="mm")
                for kc in range(NKC):
                    nc.tensor.matmul(h_ps[:], wt[:, kc, :], e1t[:, kc, :],
                                     start=(kc == 0), stop=(kc == NKC - 1))
                h_sb = hpool.tile([P, R], F32, tag="h")
                nc.scalar.activation(h_sb[:], h_ps[:], gelu_func,
                                     bias=b1t[:, f:f + 1])
                w2t = wpool.tile([P, D], F32, tag="w")
                nc.sync.dma_start(w2t[:], w2_d[f * P:(f + 1) * P, :])
                for s in range(S):
                    for h in range(NH):
                        nc.tensor.matmul(
                            o2_ps[s][:C, h * 512:(h + 1) * 512],
                            h_sb[:, s * C:(s + 1) * C],
                            w2t[:, h * 512:(h + 1) * 512],
                            start=(f == 0), stop=(f == NFF - 1),
                            skip_group_check=True)

            # ---- residual + LN2 + store -----------------------------------
            for s in range(S):
                x2 = work.tile([P, D], F32, tag="x")
                nc.vector.tensor_add(x2[:C], e1[:C, s, :], o2_ps[s][:C, :])
                nc.vector.tensor_add(x2[:C], x2[:C], b2b[:C])
                if debug:
                    nc.sync.dma_start(dbg_x2[s], x2[:C])
                fin = work.tile([P, D], F32, tag="x")
                layer_norm(fin[:C], x2[:C], g2b, be2b)
                nc.sync.dma_start(y_d[s], fin[:C])

    if not nc.is_finalized():
        nc.finalize()
    return nc


_NC_CACHE = {}


def _get_program():
    key = "full"
    if key not in _NC_CACHE:
        _NC_CACHE[key] = build_program()
    return _NC_CACHE[key]


def kernel(**inputs) -> np.ndarray:
    from concourse.bass_utils import run_bass_kernel_spmd

    nc = _get_program()
    B = inputs["x"].shape[0]
    spc = B // N_CORES  # samples per core

    weights = {k: np.ascontiguousarray(np.asarray(v, dtype=np.float32))
               for k, v in inputs.items() if k not in ("x", "mask")}
    x = np.asarray(inputs["x"], dtype=np.float32)
    mask = np.asarray(inputs["mask"], dtype=np.float32)

    in_maps = []
    for core in range(N_CORES):
        m = dict(weights)
        m["x"] = np.ascontiguousarray(x[core * spc:(core + 1) * spc])
        m["mask"] = np.ascontiguousarray(mask[core * spc:(core + 1) * spc])
        in_maps.append(m)

    res = run_bass_kernel_spmd(nc, in_maps, list(range(N_CORES)))
    return np.concatenate([res.results[k]["y"] for k in range(N_CORES)],
                          axis=0)
